# revision 20
# baseline (speedup 1.0000x reference)
"""Trainium2 Bass kernel for nn_PointNetfeat (DGCNN-style PointNet feature net).

Sharding: pure data parallel - one sample per NeuronCore (B=8, 8 cores).
Sync-BN batch statistics are exchanged with small AllReduce collectives.

Key algorithmic transformations (validated in proto.py against reference):
  - knn top-5 over pd rows == top-5 over z = X@X.T - d2/2 rows (row-constant
    offsets don't change per-row argsort). InstMax/InstMaxIndex give top-8.
  - edge conv via u/v trick: h[n,k,:] = u[idx[n,k]] - u[n] + v[n] with
    u = X@Wa.T, v = X@Wb.T, W = [Wa|Wb]; so h[n,k] = ug[n,k] + w[n], w = v-u.
    Neighbor gather of u rows via indirect DMA (k=0 gathers self: ug[:,0]=u).
  - BN(h) stats: blocks 1-3 from materialized h tiles via PE matmuls
    (sum h = last col of h^T [h|1]; sum h^2 = diag(h^T h)).
    Block 4 (O=1024) from feature-space second moments (F-trick):
    S2 = diag(W F W^T), F built from nb^T nb, S^T X, X^T X.
  - max over k (and over n for the global feature) commutes with the BN
    affine + leaky relu because a = g*rsqrt(var+eps) > 0 (g == 1).
  - conv/fc biases vanish through training-mode BN (mean subtraction).
"""
import numpy as np

N = 2048
NT = 16          # N // 128
KNN = 5
EPS = 1e-5
P = 128

_cache = {}


# ----------------------------------------------------------------- host prep
def _prep_params(params):
    f32 = lambda a: np.ascontiguousarray(np.asarray(a), dtype=np.float32)
    d = {}
    for net, kk in (("stn", 5), ("fstn", 64)):
        sp = params[net]
        for i, cp in enumerate(sp["convs"]):
            w = f32(cp["w"])
            O = w.shape[0]
            d[f"{net}_c{i}_wT"] = w.T.copy()
            if O <= P:
                d[f"{net}_c{i}_g"] = f32(cp["g"]).reshape(O, 1)
                d[f"{net}_c{i}_be"] = f32(cp["beta"]).reshape(O, 1)
            else:
                d[f"{net}_c{i}_g"] = f32(cp["g"]).reshape(O // P, P).T.copy()
                d[f"{net}_c{i}_be"] = f32(cp["beta"]).reshape(O // P, P).T.copy()
        for i in range(2):
            fp = sp["fcs"][i]
            w = f32(fp["w"])
            O = w.shape[0]
            d[f"{net}_f{i}_wT"] = w.T.copy()
            d[f"{net}_f{i}_g"] = f32(fp["g"]).reshape(O // P, P).T.copy()
            d[f"{net}_f{i}_be"] = f32(fp["beta"]).reshape(O // P, P).T.copy()
        fp = sp["fcs"][2]
        w = f32(fp["w"])
        d[f"{net}_f2_wT"] = w.T.copy()
        be = f32(fp["b"]) + np.eye(kk, dtype=np.float32).flatten()
        if w.shape[0] <= P:
            d[f"{net}_f2_b"] = be.reshape(-1, 1)
        else:
            d[f"{net}_f2_b"] = be.reshape(-1, P).T.copy()   # [128, 32]
    for i, epp in enumerate(params["edge"]):
        w = f32(epp["w"])
        O, C2 = w.shape
        C = C2 // 2
        d[f"e{i}_waT"] = w[:, :C].T.copy()
        d[f"e{i}_wbT"] = w[:, C:].T.copy()
        if O <= P:
            d[f"e{i}_g"] = f32(epp["g"]).reshape(O, 1)
            d[f"e{i}_be"] = f32(epp["beta"]).reshape(O, 1)
        else:
            d[f"e{i}_g"] = f32(epp["g"]).reshape(O // P, P).T.copy()
            d[f"e{i}_be"] = f32(epp["beta"]).reshape(O // P, P).T.copy()
    d["e3_w"] = f32(params["edge"][3]["w"])                 # [1024, 256]
    return d


# ------------------------------------------------------------------- builder
def build_program(nc, num_cores):
    import concourse.bass as bass
    import concourse.mybir as mybir
    import concourse.tile as tile
    from concourse.masks import make_identity

    f32 = mybir.dt.float32
    f16 = mybir.dt.float16
    u32 = mybir.dt.uint32
    Alu = mybir.AluOpType
    Act = mybir.ActivationFunctionType
    RG = [list(range(num_cores))]

    di = {}   # dram inputs

    def din(name, shape):
        di[name] = nc.dram_tensor(name, list(shape), f32, kind="ExternalInput")
        return di[name]

    x_in = din("x", (5, N))
    for net, kk in (("stn", 5), ("fstn", 64)):
        cins = [kk, 64, 64, 128]
        couts = [64, 64, 128, 1024]
        for i in range(4):
            din(f"{net}_c{i}_wT", (cins[i], couts[i]))
            gs = (couts[i], 1) if couts[i] <= P else (P, couts[i] // P)
            din(f"{net}_c{i}_g", gs)
            din(f"{net}_c{i}_be", gs)
        din(f"{net}_f0_wT", (1024, 512))
        din(f"{net}_f0_g", (P, 4)); din(f"{net}_f0_be", (P, 4))
        din(f"{net}_f1_wT", (512, 256))
        din(f"{net}_f1_g", (P, 2)); din(f"{net}_f1_be", (P, 2))
        din(f"{net}_f2_wT", (256, kk * kk))
        din(f"{net}_f2_b", (kk * kk, 1) if kk == 5 else (P, 32))
    ecin = [5, 64, 64, 128]
    ecout = [64, 64, 128, 1024]
    for i in range(4):
        din(f"e{i}_waT", (ecin[i], ecout[i]))
        din(f"e{i}_wbT", (ecin[i], ecout[i]))
        gs = (ecout[i], 1) if ecout[i] <= P else (P, ecout[i] // P)
        din(f"e{i}_g", gs)
        din(f"e{i}_be", gs)
    din("e3_w", (1024, 256))

    out_o = nc.dram_tensor("out_o", [1280, N], f32, kind="ExternalOutput")
    trans_o = nc.dram_tensor("trans_o", [25], f32, kind="ExternalOutput")
    tf_o = nc.dram_tensor("tf_o", [32, P], f32, kind="ExternalOutput")

    with tile.TileContext(nc) as tc:
        import contextlib
        ctx = contextlib.ExitStack()
        with ctx:
            sb = ctx.enter_context(tc.tile_pool(name="sb", bufs=1))
            sm = ctx.enter_context(tc.tile_pool(name="sm", bufs=4))
            wp = ctx.enter_context(tc.tile_pool(name="wp", bufs=1))
            ws = ctx.enter_context(tc.tile_pool(name="ws", bufs=3))
            pb2 = ctx.enter_context(tc.tile_pool(name="pb2", bufs=2))
            hp3 = ctx.enter_context(tc.tile_pool(name="hp3", bufs=3))
            dram = ctx.enter_context(tc.tile_pool(name="dram", bufs=1, space="DRAM"))
            pbig = ctx.enter_context(tc.tile_pool(name="pbig", bufs=1, space="PSUM"))
            pmed = ctx.enter_context(tc.tile_pool(name="pmed", bufs=2, space="PSUM"))
            pstat = ctx.enter_context(tc.tile_pool(name="pstat", bufs=1, space="PSUM"))

            ident = sb.tile([P, P], f32, tag="ident")
            make_identity(nc, ident)
            ones1 = sb.tile([1, P], f32, tag="ones1")
            nc.vector.memset(ones1[:], 1.0)
            onescol = sb.tile([P, 1], f32, tag="onescol")
            nc.vector.memset(onescol[:], 1.0)
            ident16 = sb.tile([P, P], f16, tag="ident16")
            nc.vector.tensor_copy(ident16[:], ident[:])

            def ts(t):
                return slice(t * P, (t + 1) * P)

            def load(name, tag=None, pool=None):
                src = di[name]
                p = pool or wp
                t = p.tile(list(src.shape), f32, tag=tag or name)
                nc.sync.dma_start(t[:], src[:])
                return t

            def allreduce(tag, src_ap, shape):
                a_in = dram.tile(list(shape), f32, tag=f"ar_{tag}_i")
                a_out = dram.tile(list(shape), f32, tag=f"ar_{tag}_o")
                nc.gpsimd.dma_start(a_in[:], src_ap)
                nc.gpsimd.collective_compute(
                    "AllReduce", Alu.add, replica_groups=RG,
                    ins=[a_in.opt()], outs=[a_out.opt()])
                res = sm.tile(list(shape), f32, tag=f"ar_{tag}_r")
                nc.gpsimd.dma_start(res[:], a_out[:])
                return res

            def bn_coeffs(tag, sum_ap, sumsq_ap, g_ap, be_ap, count, pp, nch,
                          out_pair=False):
                """a = g*rsqrt(var+eps), b = beta - mean*a; shapes [pp, nch]."""
                mean = sm.tile([pp, nch], f32, tag=f"bn_{tag}_m")
                nc.scalar.mul(mean[:], sum_ap, 1.0 / count)
                msq = sm.tile([pp, nch], f32, tag=f"bn_{tag}_q")
                nc.scalar.mul(msq[:], sumsq_ap, 1.0 / count)
                var = sm.tile([pp, nch], f32, tag=f"bn_{tag}_v")
                nc.vector.tensor_tensor(out=var[:], in0=mean[:], in1=mean[:], op=Alu.mult)
                nc.vector.tensor_tensor(out=var[:], in0=msq[:], in1=var[:], op=Alu.subtract)
                nc.vector.tensor_scalar_add(var[:], var[:], EPS)
                sq = sm.tile([pp, nch], f32, tag=f"bn_{tag}_s")
                nc.scalar.activation(sq[:], var[:], Act.Sqrt)
                y = sm.tile([pp, nch], f32, tag=f"bn_{tag}_y")
                nc.vector.reciprocal(y[:], sq[:])
                # one Newton step on rsqrt: y' = y*(1.5 - 0.5*var*y^2)
                t1 = sm.tile([pp, nch], f32, tag=f"bn_{tag}_t")
                nc.vector.tensor_tensor(out=t1[:], in0=y[:], in1=y[:], op=Alu.mult)
                nc.vector.tensor_tensor(out=t1[:], in0=var[:], in1=t1[:], op=Alu.mult)
                nc.vector.tensor_scalar_mul(t1[:], t1[:], -0.5)
                nc.vector.tensor_scalar_add(t1[:], t1[:], 1.5)
                nc.vector.tensor_tensor(out=y[:], in0=y[:], in1=t1[:], op=Alu.mult)
                if out_pair:
                    ab = sm.tile([pp, 2], f32, tag=f"bn_{tag}_ab")
                    a_ap, b_ap = ab[:, 0:1], ab[:, 1:2]
                else:
                    a_t = sm.tile([pp, nch], f32, tag=f"bn_{tag}_a")
                    b_t = sm.tile([pp, nch], f32, tag=f"bn_{tag}_b")
                    a_ap, b_ap = a_t[:], b_t[:]
                nc.vector.tensor_tensor(out=a_ap, in0=y[:], in1=g_ap, op=Alu.mult)
                nc.vector.tensor_tensor(out=t1[:], in0=mean[:], in1=a_ap, op=Alu.mult)
                nc.vector.tensor_tensor(out=b_ap, in0=be_ap, in1=t1[:], op=Alu.subtract)
                if out_pair:
                    return ab
                return a_ap, b_ap

            def psum_stats_pack(tag, hpsum_ap, nfree, dst_ap_sum, dst_ap_sq, scale, pp=P):
                ng = (nfree + 511) // 512
                bnst = sm.tile([pp, ng, 6], f32, tag=f"st_{tag}_6")
                for j in range(ng):
                    nc.vector.bn_stats(out=bnst[:, j, :], in_=hpsum_ap[:, j * 512:(j + 1) * 512])
                bnag = sm.tile([pp, 2], f32, tag=f"st_{tag}_2")
                nc.vector.bn_aggr(out=bnag[:], in_=bnst[:])
                nc.scalar.mul(dst_ap_sum, bnag[:, 0:1], float(scale))
                t1 = sm.tile([pp, 1], f32, tag=f"st_{tag}_t")
                nc.vector.tensor_tensor(out=t1[:], in0=bnag[:, 0:1], in1=bnag[:, 0:1], op=Alu.mult)
                nc.vector.tensor_tensor(out=t1[:], in0=bnag[:, 1:2], in1=t1[:], op=Alu.add)
                nc.scalar.mul(dst_ap_sq, t1[:], float(scale))

            def stream_w(dname, kt, m, rows, cols):
                """Stream [rows, cols] slice (kt*128 row offset, m*128 col offset)."""
                t = ws.tile([rows, cols], f32, tag="fcw")
                nc.sync.dma_start(
                    t[:], di[dname][kt * P:kt * P + rows, m * P:m * P + cols])
                return t

            # =============================================== STN / FSTN =====
            def stn_net(net, kk, XTin):
                cins = [kk, 64, 64, 128]
                couts = [64, 64, 128, 1024]
                h = XTin
                hv = None
                for i in range(4):
                    Ci, O = cins[i], couts[i]
                    wT = load(f"{net}_c{i}_wT", tag=f"cw{i}")
                    nch = max(1, O // P)
                    pp = min(O, P)
                    arpk = sm.tile([pp, 2 * nch], f32, tag=f"{net}c{i}ar")
                    if i == 3:
                        hvm = sm.tile([P, 8], f32, tag=f"{net}hvm")
                    hu = None
                    if i < 3:
                        hu = pb2.tile([O, N], f32, tag="hu")
                    for m in range(nch):
                        hpm = pbig.tile([pp, N], f32, tag="pbig")
                        for c in range(4):
                            nc.tensor.matmul(
                                hpm[:, c * 512:(c + 1) * 512],
                                lhsT=wT[0:Ci, m * P:m * P + pp],
                                rhs=h[0:Ci, c * 512:(c + 1) * 512],
                                start=True, stop=True)
                        psum_stats_pack(f"{net}c{i}", hpm, N,
                                        arpk[:, m:m + 1], arpk[:, nch + m:nch + m + 1],
                                        N, pp=pp)
                        if i < 3:
                            nc.scalar.copy(hu[:], hpm[0:O, :])
                        else:
                            nc.vector.tensor_reduce(out=hvm[:, m:m + 1], in_=hpm[:],
                                                    axis=mybir.AxisListType.X, op=Alu.max)
                    ar = allreduce(f"{net}c{i}", arpk[:], (pp, 2 * nch))
                    g_t = load(f"{net}_c{i}_g", tag=f"gg{i}", pool=sm)
                    be_t = load(f"{net}_c{i}_be", tag=f"bb{i}", pool=sm)
                    a_ap, b_ap = bn_coeffs(f"{net}c{i}", ar[:, 0:nch], ar[:, nch:2 * nch],
                                           g_t[:], be_t[:], num_cores * N, pp, nch)
                    if i < 3:
                        nc.scalar.activation(hu[:], hu[:], Act.Relu,
                                             bias=b_ap[:, 0:1], scale=a_ap[:, 0:1])
                        h = hu
                    else:
                        hv = sm.tile([P, 8], f32, tag=f"{net}hv")
                        nc.vector.tensor_tensor(out=hv[:], in0=hvm[:], in1=a_ap, op=Alu.mult)
                        nc.vector.tensor_tensor(out=hv[:], in0=hv[:], in1=b_ap, op=Alu.add)
                        nc.vector.tensor_scalar_max(hv[:], hv[:], 0.0)
                # FC1 [1024]->[512], FC2 [512]->[256]  (batch BN, relu)
                vec = hv
                for fi, (Ki, O) in enumerate(((1024, 512), (512, 256))):
                    nkt, nmc = Ki // P, O // P
                    fp_ = pmed.tile([P, nmc], f32, tag="pmed")
                    for m in range(nmc):
                        for kt in range(nkt):
                            wt = stream_w(f"{net}_f{fi}_wT", kt, m, P, P)
                            nc.tensor.matmul(
                                fp_[:, m:m + 1], lhsT=wt[:], rhs=vec[:, kt:kt + 1],
                                start=(kt == 0), stop=(kt == nkt - 1))
                    hloc = sm.tile([P, nmc], f32, tag=f"{net}f{fi}h")
                    nc.scalar.copy(hloc[:], fp_[:])
                    arpk = sm.tile([P, 2 * nmc], f32, tag=f"{net}f{fi}ar")
                    nc.vector.tensor_copy(arpk[:, 0:nmc], hloc[:])
                    nc.vector.tensor_tensor(out=arpk[:, nmc:2 * nmc], in0=hloc[:],
                                            in1=hloc[:], op=Alu.mult)
                    ar = allreduce(f"{net}f{fi}", arpk[:], (P, 2 * nmc))
                    g_t = load(f"{net}_f{fi}_g", tag=f"fg{fi}", pool=sm)
                    be_t = load(f"{net}_f{fi}_be", tag=f"fb{fi}", pool=sm)
                    a_ap, b_ap = bn_coeffs(f"{net}f{fi}", ar[:, 0:nmc], ar[:, nmc:2 * nmc],
                                           g_t[:], be_t[:], num_cores, P, nmc)
                    yv = sm.tile([P, nmc], f32, tag=f"{net}f{fi}y")
                    nc.vector.tensor_tensor(out=yv[:], in0=hloc[:], in1=a_ap, op=Alu.mult)
                    nc.vector.tensor_tensor(out=yv[:], in0=yv[:], in1=b_ap, op=Alu.add)
                    nc.vector.tensor_scalar_max(yv[:], yv[:], 0.0)
                    vec = yv
                # FC3 + bias(+I)
                b_t = load(f"{net}_f2_b", tag="f2b", pool=sm)
                if kk == 5:
                    f3p = pmed.tile([25, 1], f32, tag="pmed")
                    for kt in range(2):
                        wt = stream_w(f"{net}_f2_wT", kt, 0, P, 25)
                        nc.tensor.matmul(f3p[:], lhsT=wt[:], rhs=vec[:, kt:kt + 1],
                                         start=(kt == 0), stop=(kt == 1))
                    t25 = sm.tile([25, 1], f32, tag="t25")
                    nc.vector.tensor_tensor(out=t25[:], in0=f3p[:], in1=b_t[:], op=Alu.add)
                    nc.sync.dma_start(trans_o[:], t25[:, 0:1])
                    return None
                else:
                    f3p = pmed.tile([P, 32], f32, tag="pmed")
                    for m in range(32):
                        for kt in range(2):
                            wt = stream_w(f"{net}_f2_wT", kt, m, P, P)
                            nc.tensor.matmul(
                                f3p[:, m:m + 1], lhsT=wt[:], rhs=vec[:, kt:kt + 1],
                                start=(kt == 0), stop=(kt == 1))
                    tfs = sm.tile([P, 32], f32, tag="tfs")
                    nc.vector.tensor_tensor(out=tfs[:], in0=f3p[:], in1=b_t[:], op=Alu.add)
                    nc.sync.dma_start(tf_o[:].rearrange("j p -> p j"), tfs[:])
                    tf2 = sb.tile([64, 64], f32, tag="tf2")
                    nc.sync.dma_start(
                        tf2[:], tf_o[:].rearrange("j p -> (j p)").rearrange("(r c) -> r c", c=64))
                    return tf2

            # ================================================ EDGE BLOCK ====
            def edge_block(bi, XT, C, O, XTnext, Xpm4=None, x33_dram=None):
                b4 = (bi == 3)
                waT = load(f"e{bi}_waT")
                wbT = load(f"e{bi}_wbT")
                # ---- d2 row
                negd2 = sb.tile([1, N], f32, tag="nd2")
                for c in range(4):
                    sqc = pb2.tile([P, 512], f32, tag="sqc")
                    nc.scalar.activation(sqc[0:C, :], XT[0:C, c * 512:(c + 1) * 512],
                                         Act.Square)
                    dp = pmed.tile([1, 512], f32, tag="pmed")
                    nc.tensor.matmul(dp[:], lhsT=onescol[0:C, :], rhs=sqc[0:C, :],
                                     start=True, stop=True)
                    nc.scalar.activation(negd2[:, c * 512:(c + 1) * 512], dp[:], Act.Copy,
                                         scale=-0.5)
                # ---- u, v, w
                udt = f16 if b4 else f32
                u_dram = dram.tile([N, O], udt, tag=f"u{bi}")
                if b4:
                    w_dram = dram.tile([N, O], f16, tag="w4d")
                else:
                    wfull = sb.tile([P, NT, O], f32, tag="wf")
                for t in range(NT):
                    if b4:
                        uvp = pbig.tile([P, 2048], f32, tag="pbig")
                        for c in range(2):
                            nc.tensor.matmul(uvp[:, c * 512:(c + 1) * 512],
                                             lhsT=XT[0:C, ts(t)],
                                             rhs=waT[:, c * 512:(c + 1) * 512],
                                             start=True, stop=True)
                            nc.tensor.matmul(uvp[:, 1024 + c * 512:1024 + (c + 1) * 512],
                                             lhsT=XT[0:C, ts(t)],
                                             rhs=wbT[:, c * 512:(c + 1) * 512],
                                             start=True, stop=True)
                        us = pb2.tile([P, 1024], udt, tag="us")
                        nc.scalar.copy(us[:, 0:O], uvp[:, 0:1024])
                        nc.sync.dma_start(u_dram[ts(t), :], us[:, 0:O])
                        wt4 = pb2.tile([P, 1024], f16, tag="mx4")
                        nc.vector.tensor_tensor(out=wt4[:], in0=uvp[:, 1024:2048],
                                                in1=us[:, 0:O], op=Alu.subtract)
                        nc.sync.dma_start(w_dram[ts(t), :], wt4[:])
                    else:
                        uvp = pmed.tile([P, 2 * O], f32, tag="pmed")
                        nc.tensor.matmul(uvp[:, 0:O], lhsT=XT[0:C, ts(t)], rhs=waT[:],
                                         start=True, stop=True)
                        nc.tensor.matmul(uvp[:, O:2 * O], lhsT=XT[0:C, ts(t)], rhs=wbT[:],
                                         start=True, stop=True)
                        us = pb2.tile([P, 1024], udt, tag="us")
                        nc.scalar.copy(us[:, 0:O], uvp[:, 0:O])
                        nc.sync.dma_start(u_dram[ts(t), :], us[:, 0:O])
                        nc.vector.tensor_tensor(out=wfull[:, t, :], in0=uvp[:, O:2 * O],
                                                in1=us[:, 0:O], op=Alu.subtract)
                # ---- z + knn + gather + per-tile stats matmuls + max tree
                if not b4:
                    qfull = sb.tile([P, NT, O], f32, tag="qsf")
                    statp = pstat.tile([O, O + 1], f32, tag="pstat")
                else:
                    sfull = sb.tile([P, NT, C], f32, tag="qsf")
                    gq = sb.tile([P, 1024], f16, tag="gq4")
                    f1p = pstat.tile([C, C], f32, tag="pstat")
                for t in range(NT):
                    zp = pbig.tile([P, N], f32, tag="pbig")
                    for c in range(4):
                        cs = slice(c * 512, (c + 1) * 512)
                        nc.tensor.matmul(zp[:, cs], lhsT=XT[0:C, ts(t)], rhs=XT[0:C, cs],
                                         start=True, stop=False)
                        nc.tensor.matmul(zp[:, cs], lhsT=ones1[:, 0:P], rhs=negd2[:, cs],
                                         start=False, stop=True)
                    m8 = sm.tile([P, 8], f32, tag="m8")
                    nc.vector.max(out=m8[:], in_=zp[:])
                    zs = pb2.tile([P, N], f32, tag="zs")
                    nc.scalar.copy(zs[:], zp[:])
                    i8 = hp3.tile([P, 8], u32, tag="i8")
                    nc.vector.max_index(out=i8[:], in_max=m8[:], in_values=zs[:])
                    if not b4:
                        ht = hp3.tile([P, KNN, O + 1], f32, tag="ht")
                        nc.vector.memset(ht[:, :, O:O + 1], 1.0)
                        for k in range(KNN):
                            nc.gpsimd.indirect_dma_start(
                                out=ht[:, k, 0:O], out_offset=None, in_=u_dram[:],
                                in_offset=bass.IndirectOffsetOnAxis(ap=i8[:, k:k + 1], axis=0))
                        nc.vector.tensor_tensor(
                            out=ht[:, :, 0:O], in0=ht[:, :, 0:O],
                            in1=wfull[:, t:t + 1, :].to_broadcast([P, KNN, O]), op=Alu.add)
                        for k in range(KNN):
                            nc.tensor.matmul(statp[:], lhsT=ht[:, k, 0:O], rhs=ht[:, k, :],
                                             start=(t == 0 and k == 0),
                                             stop=(t == NT - 1 and k == KNN - 1),
                                             skip_group_check=True)
                        tA = sm.tile([P, O], f32, tag="tA")
                        tB = sm.tile([P, O], f32, tag="tB")
                        nc.vector.tensor_tensor(out=tA[:], in0=ht[:, 0, 0:O],
                                                in1=ht[:, 1, 0:O], op=Alu.max)
                        nc.vector.tensor_tensor(out=tB[:], in0=ht[:, 2, 0:O],
                                                in1=ht[:, 3, 0:O], op=Alu.max)
                        nc.vector.tensor_tensor(out=tA[:], in0=tA[:], in1=tB[:], op=Alu.max)
                        nc.vector.tensor_tensor(out=qfull[:, t, :], in0=tA[:],
                                                in1=ht[:, 4, 0:O], op=Alu.max)
                    else:
                        mx = pb2.tile([P, 1024], f16, tag="mx4")
                        nbt = hp3.tile([P, KNN, C], f32, tag="ht")
                        for k in range(KNN):
                            gk = pb2.tile([P, 1024], f16, tag="gk")
                            nc.gpsimd.indirect_dma_start(
                                out=gk[:], out_offset=None, in_=u_dram[:],
                                in_offset=bass.IndirectOffsetOnAxis(ap=i8[:, k:k + 1], axis=0))
                            if k == 0:
                                nc.vector.tensor_copy(mx[:], gk[:])
                            else:
                                nc.vector.tensor_tensor(out=mx[:], in0=mx[:], in1=gk[:],
                                                        op=Alu.max)
                            nc.gpsimd.indirect_dma_start(
                                out=nbt[:, k, :], out_offset=None, in_=x33_dram[:],
                                in_offset=bass.IndirectOffsetOnAxis(ap=i8[:, k:k + 1], axis=0))
                        for k in range(KNN):
                            nc.tensor.matmul(f1p[:], lhsT=nbt[:, k, :], rhs=nbt[:, k, :],
                                             start=(t == 0 and k == 0),
                                             stop=(t == NT - 1 and k == KNN - 1),
                                             skip_group_check=True)
                        nc.vector.tensor_tensor(out=sfull[:, t, :], in0=nbt[:, 0, :],
                                                in1=nbt[:, 1, :], op=Alu.add)
                        for k in range(2, KNN):
                            nc.vector.tensor_tensor(out=sfull[:, t, :], in0=sfull[:, t, :],
                                                    in1=nbt[:, k, :], op=Alu.add)
                        # q16 = maxu + w ; global running max into gq
                        wl = pb2.tile([P, 1024], f16, tag="w4t")
                        nc.sync.dma_start(wl[:], w_dram[ts(t), :])
                        nc.vector.tensor_tensor(out=mx[:], in0=mx[:], in1=wl[:],
                                                op=Alu.add)
                        if t == 0:
                            nc.vector.tensor_copy(gq[:], mx[:])
                        else:
                            nc.vector.tensor_tensor(out=gq[:], in0=gq[:], in1=mx[:], op=Alu.max)
                # ---- stats finish + y / x4
                cnt_all = float(num_cores * N * KNN)
                if not b4:
                    s1s2 = sm.tile([O, 2], f32, tag="ss")
                    nc.vector.tensor_copy(s1s2[:, 0:1], statp[:, O:O + 1])
                    dg = sm.tile([O, O], f32, tag="dgd")
                    nc.vector.tensor_tensor(out=dg[:], in0=statp[:, 0:O],
                                            in1=ident[0:O, 0:O], op=Alu.mult)
                    nc.vector.tensor_reduce(out=s1s2[:, 1:2], in_=dg[:],
                                            axis=mybir.AxisListType.X, op=Alu.add)
                    ar = allreduce(f"e{bi}", s1s2[:], (O, 2))
                    g_t = load(f"e{bi}_g", tag="eg", pool=sm)
                    be_t = load(f"e{bi}_be", tag="ebe", pool=sm)
                    ab = bn_coeffs(f"e{bi}", ar[:, 0:1], ar[:, 1:2], g_t[:], be_t[:],
                                   cnt_all, O, 1, out_pair=True)
                    arow = sm.tile([1, O], f32, tag="abrA")
                    brow = sm.tile([1, O], f32, tag="abrB")
                    for j, row in enumerate((arow, brow)):
                        trp = pmed.tile([P, P], f32, tag="pmed")
                        nc.tensor.transpose(out=trp[0:1, 0:O], in_=ab[:, j:j + 1],
                                            identity=ident[0:O, 0:O])
                        nc.scalar.copy(row[:], trp[0:1, 0:O])
                    abc = sb.tile([P, 2, O], f32, tag="abc")
                    for j, row in enumerate((arow, brow)):
                        bp = pmed.tile([P, O], f32, tag="pmed")
                        nc.tensor.matmul(bp[:], lhsT=ones1[:, 0:P], rhs=row[:],
                                         start=True, stop=True)
                        nc.scalar.copy(abc[:, j, :], bp[:])
                    for t in range(NT):
                        if bi == 2:
                            yt = Xpm4[:, t, 0:P]
                        else:
                            ytt = hp3.tile([P, O], f32, tag="yt")
                            yt = ytt[:]
                        nc.vector.tensor_tensor(out=yt, in0=qfull[:, t, :], in1=abc[:, 0, :],
                                                op=Alu.mult)
                        nc.vector.tensor_tensor(out=yt, in0=yt, in1=abc[:, 1, :], op=Alu.add)
                        nc.vector.scalar_tensor_tensor(out=yt, in0=yt, scalar=0.2, in1=yt,
                                                       op0=Alu.mult, op1=Alu.max)
                        trp2 = pmed.tile([P, P], f32, tag="pmed")
                        nc.tensor.transpose(out=trp2[0:O, 0:P], in_=yt, identity=ident[:])
                        nc.scalar.copy(XTnext[:, ts(t)], trp2[0:O, 0:P])
                        if bi == 2:
                            nc.sync.dma_start(x33_dram[ts(t), :], yt)
                else:
                    nbTnb = sm.tile([C, C], f32, tag="nbTnb")
                    nc.vector.tensor_copy(nbTnb[:], f1p[:])
                    f2p = pstat.tile([C, C + 1], f32, tag="pstat")
                    for t in range(NT):
                        nc.tensor.matmul(f2p[:], lhsT=sfull[:, t, :], rhs=Xpm4[:, t, :],
                                         start=(t == 0), stop=(t == NT - 1))
                    sx = sm.tile([C, C + 1], f32, tag="sx")
                    nc.vector.tensor_copy(sx[:], f2p[:])
                    f3p2 = pstat.tile([C, C + 1], f32, tag="pstat")
                    for t in range(NT):
                        nc.tensor.matmul(f3p2[:], lhsT=Xpm4[:, t, 0:C], rhs=Xpm4[:, t, :],
                                         start=(t == 0), stop=(t == NT - 1))
                    xx = sm.tile([C, C + 1], f32, tag="xx")
                    nc.vector.tensor_copy(xx[:], f3p2[:])
                    trp = pmed.tile([P, P], f32, tag="pmed")
                    nc.tensor.transpose(out=trp[0:C, 0:C], in_=sx[:, 0:C], identity=ident[:])
                    sxT = sm.tile([C, C], f32, tag="sxT")
                    nc.scalar.copy(sxT[:], trp[0:C, 0:C])
                    Ftop = sb.tile([C, 256], f32, tag="Ftop")
                    Fbot = sb.tile([C, 256], f32, tag="Fbot")
                    nc.vector.tensor_tensor(out=Ftop[:, 0:C], in0=nbTnb[:], in1=sx[:, 0:C],
                                            op=Alu.subtract)
                    nc.vector.tensor_tensor(out=Ftop[:, 0:C], in0=Ftop[:, 0:C], in1=sxT[:],
                                            op=Alu.subtract)
                    nc.vector.scalar_tensor_tensor(out=Ftop[:, 0:C], in0=xx[:, 0:C],
                                                   scalar=float(KNN), in1=Ftop[:, 0:C],
                                                   op0=Alu.mult, op1=Alu.add)
                    nc.vector.scalar_tensor_tensor(out=Ftop[:, C:256], in0=xx[:, 0:C],
                                                   scalar=-float(KNN), in1=sx[:, 0:C],
                                                   op0=Alu.mult, op1=Alu.add)
                    nc.vector.scalar_tensor_tensor(out=Fbot[:, 0:C], in0=xx[:, 0:C],
                                                   scalar=-float(KNN), in1=sxT[:],
                                                   op0=Alu.mult, op1=Alu.add)
                    nc.vector.tensor_scalar_mul(Fbot[:, C:256], xx[:, 0:C], float(KNN))
                    sfa = sm.tile([C, 1], f32, tag="sfa")
                    nc.vector.scalar_tensor_tensor(out=sfa[:], in0=xx[:, C:C + 1],
                                                   scalar=-float(KNN), in1=sx[:, C:C + 1],
                                                   op0=Alu.mult, op1=Alu.add)
                    sfb = sm.tile([C, 1], f32, tag="sfb")
                    nc.vector.tensor_scalar_mul(sfb[:], xx[:, C:C + 1], float(KNN))
                    s12 = sm.tile([P, 16], f32, tag="s12_4")
                    for m in range(8):
                        sp_ = pmed.tile([P, 1], f32, tag="pmed")
                        nc.tensor.matmul(sp_[:], lhsT=waT[:, m * P:(m + 1) * P], rhs=sfa[:],
                                         start=True, stop=False)
                        nc.tensor.matmul(sp_[:], lhsT=wbT[:, m * P:(m + 1) * P], rhs=sfb[:],
                                         start=False, stop=True)
                        nc.vector.tensor_copy(s12[:, m:m + 1], sp_[:])
                        gp = pmed.tile([P, 256], f32, tag="pmed")
                        nc.tensor.matmul(gp[:], lhsT=waT[:, m * P:(m + 1) * P], rhs=Ftop[:],
                                         start=True, stop=False)
                        nc.tensor.matmul(gp[:], lhsT=wbT[:, m * P:(m + 1) * P], rhs=Fbot[:],
                                         start=False, stop=True)
                        wmt = stream_w("e3_w", m, 0, P, 256)
                        gt = sm.tile([P, 256], f32, tag="gt4")
                        nc.vector.tensor_tensor(out=gt[:], in0=gp[:], in1=wmt[:], op=Alu.mult)
                        nc.vector.tensor_reduce(out=s12[:, 8 + m:9 + m], in_=gt[:],
                                                axis=mybir.AxisListType.X, op=Alu.add)
                    ar = allreduce("e3", s12[:], (P, 16))
                    g_t = load("e3_g", tag="eg", pool=sm)
                    be_t = load("e3_be", tag="ebe", pool=sm)
                    a_ap, b_ap = bn_coeffs("e3", ar[:, 0:8], ar[:, 8:16], g_t[:], be_t[:],
                                           cnt_all, P, 8)
                    gcol = sm.tile([P, 8], f32, tag="gcol")
                    for j in range(8):
                        tp2 = pmed.tile([P, P], f16, tag="pmed")
                        nc.tensor.transpose(out=tp2[:], in_=gq[:, j * P:(j + 1) * P],
                                            identity=ident16[:])
                        nc.vector.tensor_reduce(out=gcol[:, j:j + 1], in_=tp2[:],
                                                axis=mybir.AxisListType.X, op=Alu.max)
                    x4c = sm.tile([P, 8], f32, tag="x4c")
                    nc.vector.tensor_tensor(out=x4c[:], in0=gcol[:], in1=a_ap, op=Alu.mult)
                    nc.vector.tensor_tensor(out=x4c[:], in0=x4c[:], in1=b_ap, op=Alu.add)
                    nc.vector.scalar_tensor_tensor(out=x4c[:], in0=x4c[:], scalar=0.2,
                                                   in1=x4c[:], op0=Alu.mult, op1=Alu.max)
                    for j in range(8):
                        for c in range(4):
                            xbc = pb2.tile([P, 512], f32, tag="xbc")
                            nc.scalar.copy(xbc[:], x4c[:, j:j + 1].to_broadcast([P, 512]))
                            nc.sync.dma_start(
                                out_o[256 + j * P:256 + (j + 1) * P, c * 512:(c + 1) * 512],
                                xbc[:])

            # ================================================== wiring ======
            XT1 = sb.tile([P, N], f32, tag="XT14")
            nc.sync.dma_start(XT1[0:5, :], x_in[:])

            stn_net("stn", 5, XT1)

            x1T = sb.tile([64, N], f32, tag="x1T")
            edge_block(0, XT1, 5, 64, x1T)

            tf2 = stn_net("fstn", 64, x1T)

            XT2 = pb2.tile([64, N], f32, tag="hu")
            for c in range(4):
                cs = slice(c * 512, (c + 1) * 512)
                xp = pmed.tile([64, 512], f32, tag="pmed")
                nc.tensor.matmul(xp[:], lhsT=tf2[:], rhs=x1T[:, cs], start=True, stop=True)
                nc.scalar.copy(XT2[:, cs], xp[:])
            nc.sync.dma_start(out_o[0:64, :], XT2[:])

            XT3 = pb2.tile([64, N], f32, tag="hu")
            edge_block(1, XT2, 64, 64, XT3)
            nc.sync.dma_start(out_o[64:128, :], XT3[:])

            XT4 = sb.tile([P, N], f32, tag="XT14")
            Xpm4 = sb.tile([P, NT, 129], f32, tag="Xpm4")
            nc.vector.memset(Xpm4[:, :, 128:129], 1.0)
            x33_dram = dram.tile([N, 128], f32, tag="x33d")
            edge_block(2, XT3, 64, 128, XT4, Xpm4=Xpm4, x33_dram=x33_dram)
            nc.sync.dma_start(out_o[128:256, :], XT4[:])

            edge_block(3, XT4, 128, 1024, None, Xpm4=Xpm4, x33_dram=x33_dram)

    return di


# -------------------------------------------------------------------- runner
def _build(num_cores):
    key = ("prog", num_cores)
    if key in _cache:
        return _cache[key]
    import concourse.bacc as bacc
    nc = bacc.Bacc("TRN2", target_bir_lowering=False, debug=False,
                   enable_asserts=True, num_devices=num_cores)
    build_program(nc, num_cores)
    nc.compile()
    _cache[key] = nc
    return nc


def kernel(x, params):
    import jax
    x = np.asarray(x, dtype=np.float32)
    params = jax.tree_util.tree_map(lambda t: np.asarray(t, dtype=np.float32), params)
    B = x.shape[0]
    pins = _prep_params(params)
    nc = _build(B)
    in_maps = []
    for b in range(B):
        m = dict(pins)
        m["x"] = np.ascontiguousarray(x[b])
        in_maps.append(m)
    from concourse.bass_utils import run_bass_kernel_spmd
    res = run_bass_kernel_spmd(nc, in_maps, core_ids=list(range(B)),
                               trace=globals().get("KERNEL_TRACE", False))
    _cache["last_result"] = res
    outs = np.stack([res.results[b]["out_o"] for b in range(B)])
    trans = np.stack([res.results[b]["trans_o"].reshape(5, 5) for b in range(B)])
    tf = np.stack([res.results[b]["tf_o"].reshape(4096).reshape(64, 64) for b in range(B)])
    return outs, trans, tf
